# revision 1
# baseline (speedup 1.0000x reference)
"""GreedySampler kernel for 8 Trainium2 NeuronCores.

fp8 screen on device + exact host rescore of near-max candidates
(argmax(softmax(log(...))) = argmax(logits); fp8 logit error <=0.43
unscaled vs DELTA=2.0, so quantization only shortlists candidates).

Per core (SPMD, vocab-sharded, ragged 9x640+1x528 = 6288 cols):
  * Host packs the W shard into SBUF consumption order as one
    [P, bytes] partition-major tensor: all DMA chunks contiguous per
    partition (multi-KB descriptors; the naive strided layout's 512B
    descriptors cap at ~272GB/s, packed sustains ~320GB/s).
  * All W on the sync HWDGE ring in 0.5-1.3MB chunks (each dma_start
    costs ~600ns of HWDGE issue; the scalar ring starves under load;
    balanced dual-ring reaches 375GB/s but slows the PE ~20% via SBUF
    write contention - net loss).
  * hst and mid-stream output DMAs ride the gpsimd SWDGE ring, whose
    completion sems live outside the 8 round-robin HWDGE lanes, so
    late completions cannot block W DMA issue; the last group's
    output uses the then-idle scalar ring.
  * kk-outer accumulation over 5 concurrent PSUM banks (groups cannot
    share a 2KB bank); fine W chunks at the start (early PE start
    while cold) and end (small post-stream lag).
  * The 800 fp8 DoubleRow (LDWEIGHTS+MATMUL) pairs stream at
    ~86-92ns, the N=200 issue floor; fp32 PSUM accumulate, fp8 out.

Walrus notes: instructions carrying >1 sync wait are rejected by this
build, so excess waits are split onto preceding nops; DoubleRow lhsT
strides must be 16B-aligned (last group width 528, not 523).
"""

import math

import numpy as np
import ml_dtypes

import concourse.bass as bass
import concourse.mybir as mybir
import concourse.tile as tile
from concourse.vector_clock import ScopedClock
from concourse.bass_utils import run_bass_kernel_spmd

P = 128
N_CORES = 8
D = 4096
KK = D // 256  # 16 DoubleRow K-chunks of 256
W_SCALE = 32.0
DELTA = 2.0 * W_SCALE  # candidate margin in scaled-logit units

VGS = [640] * 9 + [528]   # ragged vocab-group widths per core
VS_EFF = sum(VGS)         # 6288
V_PAD = VS_EFF * N_CORES  # 50304 >= 50257

FP8 = mybir.dt.float8e4
F32 = mybir.dt.float32

_drain_patched = False


def _patch_tile_drain():
    """Split the tail Drain's sync waits (>1 rejected by this walrus)."""
    global _drain_patched
    if _drain_patched:
        return

    def _drain_and_barrier(self, tick_clock, wait_clock):
        nc = self.nc
        drain_inst = nc.sync.drain()
        wait_clock.add_sem_waits(
            drain_inst.ins, ScopedClock({None: tick_clock.global_clock})
        )
        si = drain_inst.ins.sync_info
        if si is not None and si.on_wait and len(si.on_wait) > 1:
            extra = list(si.on_wait[1:])
            del si.on_wait[1:]
            name2sem = {
                getattr(s, "name", None): s
                for s in self.sems.allocated().values()
            }
            for w in extra:
                nc.sync.wait_ge(name2sem[w.ant_name], w.wait_value)
        nc.all_engine_barrier()
        popped = nc._tile_sem_poison_stack.pop()
        assert popped is self._sem_poison
        nc.clear_and_free_semaphores(list(self.sems.allocated().values()))
        nc.all_engine_barrier()

    tile.TileContext._drain_and_barrier = _drain_and_barrier
    _drain_patched = True


def _split_excess_waits(nc, limit=1):
    """Move all but `limit` sync waits of every instruction onto nops
    inserted immediately before it on the same engine queue."""
    fn = nc.m.functions[0]
    for bb in fn.blocks:
        if not any(
            getattr(i, "sync_info", None) is not None
            and i.sync_info.on_wait
            and len(i.sync_info.on_wait) > limit
            for i in bb.instructions
        ):
            continue
        cur = nc.cur_bb.bb if hasattr(nc.cur_bb, "bb") else nc.cur_bb
        new_insts = []
        for inst in bb.instructions:
            si = getattr(inst, "sync_info", None)
            if si is not None and si.on_wait and len(si.on_wait) > limit:
                extra = list(si.on_wait[:-limit])
                del si.on_wait[: len(si.on_wait) - limit]
                for w in extra:
                    nop = nc.engines[inst.engine].nop(nofuse=True).ins
                    popped = cur.instructions.pop()  # nop() self-appended
                    assert popped is nop
                    nop.sync_info = mybir.SyncInfo(on_wait=[w], on_update=[])
                    new_insts.append(nop)
            new_insts.append(inst)
        bb.instructions[:] = new_insts


def _sub_widths(w):
    subs = [P] * (w // P)
    if w % P:
        subs.append(w % P)
    return subs


def build_nc(J, vgs=VGS):
    _patch_tile_drain()
    total = KK * 2 * sum(vgs)

    nc = bass.Bass()
    hst = nc.dram_tensor("hst", [P, KK, 2, J], FP8, kind="ExternalInput")
    wt = nc.dram_tensor("wt", [P, total], FP8, kind="ExternalInput")
    nsub_max = max(len(_sub_widths(w)) for w in vgs)
    logits_t = nc.dram_tensor("logits_t", [len(vgs), P, nsub_max * J], FP8,
                              kind="ExternalOutput")

    with tile.TileContext(nc) as tc:
        with (
            tc.tile_pool(name="hs", bufs=1) as hs_pool,
            tc.tile_pool(name="w", bufs=6) as w_pool,
            tc.tile_pool(name="out", bufs=4) as out_pool,
            tc.tile_pool(name="ps", bufs=8, space=bass.MemorySpace.PSUM) as ps_pool,
        ):
            # hst on the gpsimd SWDGE ring: off the sync ring (whose
            # serial order would delay every W byte) and off the scalar
            # ring (which HW-starves vs sync, poisoning the 8-lane DMA
            # sem round-robin). 2 pieces so early kk rows land first.
            hst_sb = hs_pool.tile([P, KK, 2, J], FP8)
            for sl in (slice(0, 2), slice(2, 8), slice(8, KK)):
                nc.gpsimd.dma_start(hst_sb[:, sl], hst[:, sl])

            # PE warmup: dummy DoubleRow pairs on memset tiles fill the
            # DMA-latency window before the first real pair, so the HAM
            # clock gate unthrottles (1.2->2.4GHz needs ~3.4us of PE
            # activity) before real work arrives
            wu_w = out_pool.tile([P, 2, P], FP8, name="wu_w")
            wu_h = out_pool.tile([P, 2, J], FP8, name="wu_h")
            nc.vector.memset(wu_w[:], 0.0)
            nc.vector.memset(wu_h[:], 0.0)

            # out-DMAs are batched: every HWDGE DMA occupies one of 8
            # round-robin completion-sem lanes, and a late-completing
            # out-DMA on a lane blocks the W DMA 8 positions later
            if len(vgs) == 10:
                ogroups = [(0, 4), (4, 4), (8, 1), (9, 1)]
            else:
                ogroups = [(v, 1) for v in range(len(vgs))]
            group_of = {}
            for gi, (a, n) in enumerate(ogroups):
                for v in range(a, a + n):
                    group_of[v] = gi
            ot = None

            nsubs = {wv: len(_sub_widths(wv)) for wv in set(vgs)}
            off = 0
            nch = 0
            for vg, wv in enumerate(vgs):
                subs = _sub_widths(wv)
                w_sb = w_pool.tile([P, KK, 2, wv], FP8, name="w_sb")
                # W chunks alternate between the two HWDGE rings (sync
                # and scalar): one ring under 8-core load sustains only
                # ~300GB/s; two rings reach ~375GB/s (HW-measured).
                # Both ring queues carry ONLY W DMAs - any PE-dependent
                # instruction there would block later DMA issues.
                # Fine chunks at the start (fast PE start) and end
                # (small post-stream lag); halves otherwise (each
                # dma_start costs ~600ns HWDGE issue time).
                if vg == 0:
                    kk_cuts = [0, 4, 8, KK]
                elif vg == len(vgs) - 1:
                    kk_cuts = [0, 8, 12, 14, KK]
                else:
                    kk_cuts = [0, 8, KK]
                for a, b in zip(kk_cuts[:-1], kk_cuts[1:]):
                    src = wt[:, off + a * 2 * wv: off + b * 2 * wv]
                    # all W on the sync ring: the scalar ring is starved
                    # under load (its chunks complete late and stall the
                    # PE), and balanced dual-ring slows the PE ~20% via
                    # SBUF write contention
                    nc.sync.dma_start(
                        w_sb[:, a:b],
                        src.rearrange("p (k t w) -> p k t w", k=b - a, t=2),
                    )
                    nch += 1

                gi = group_of[vg]
                ga, gn = ogroups[gi]
                if vg == ga:
                    ot = out_pool.tile([P, gn, nsubs[wv], J], FP8, name="ot")
                # one 2KB PSUM bank per sub: concurrent accumulation
                # groups cannot share a bank (zero region)
                pss = [ps_pool.tile([P, 512], F32, name="ps") for _ in subs]
                if vg == 0:
                    # complete (start+stop) dummy groups; the bank is
                    # free again before the real kk=0 accumulation
                    for _ in range(28):
                        nc.tensor.matmul(
                            pss[0][:, :J], wu_w[:], wu_h[:],
                            start=True, stop=True,
                            perf_mode=mybir.MatmulPerfMode.DoubleRow,
                        )
                for kk in range(KK):
                    soff = 0
                    for s, sw in enumerate(subs):
                        nc.tensor.matmul(
                            pss[s][:sw, :J],
                            w_sb[:, kk, :, soff:soff + sw],
                            hst_sb[:, kk, :, :],
                            start=(kk == 0),
                            stop=(kk == KK - 1),
                            perf_mode=mybir.MatmulPerfMode.DoubleRow,
                        )
                        soff += sw
                # fp8 copies on DVE only mid-stream (the scalar SEQ
                # must stay free for its W ring); the post-stream last
                # group splits DVE/ACT so the tail drains in parallel
                last = vg == len(vgs) - 1
                for s, sw in enumerate(subs):
                    if last and s % 2 == 1:
                        nc.scalar.copy(ot[:sw, vg - ga, s, :],
                                       pss[s][:sw, :J])
                    else:
                        nc.vector.tensor_copy(ot[:sw, vg - ga, s, :],
                                              pss[s][:sw, :J])
                if vg == ga + gn - 1:
                    # mid-stream groups ship via gpsimd (SWDGE has its
                    # own completion-sem lanes, so a late out cannot
                    # block the W rings' 8-lane round-robin); the last
                    # group ships via scalar, whose ring is free once
                    # the W stream has ended
                    nfull = sum(1 for sw in subs if sw == P)
                    if nfull == len(subs):
                        nc.gpsimd.dma_start(
                            logits_t[ga:ga + gn].rearrange("v p x -> p v x"),
                            ot[:].rearrange("p v s j -> p v (s j)"),
                        )
                    else:
                        nc.scalar.dma_start(
                            logits_t[vg, :, :nfull * J],
                            ot[:, 0, :nfull, :].rearrange("p s j -> p (s j)"),
                        )
                        sw = subs[-1]
                        nc.scalar.dma_start(
                            logits_t[vg, :sw, nfull * J:(nfull + 1) * J],
                            ot[:sw, 0, nfull, :],
                        )
                off += KK * 2 * wv

    _split_excess_waits(nc, limit=1)
    return nc


def _pack_w(shard, vgs=VGS):
    """shard [D, VS_EFF] fp8 -> [P, KK*2*VS_EFF] partition-major,
    vg-blocked, contiguous in DMA consumption order."""
    blocks = []
    off = 0
    for wv in vgs:
        a = shard[:, off:off + wv].reshape(KK, 2, P, wv)
        blocks.append(np.ascontiguousarray(
            a.transpose(2, 0, 1, 3)).reshape(P, -1))
        off += wv
    return np.concatenate(blocks, axis=1)


def _decode_logits(out, vgs, J):
    """[NVG, P, nsub_max*J] fp8 -> [VS_EFF, J] f32."""
    nvg = len(vgs)
    nsub_max = out.shape[2] // J
    res = np.empty((sum(vgs), J), np.float32)
    off = 0
    o = out.astype(np.float32).reshape(nvg, P, nsub_max, J)
    for vg, wv in enumerate(vgs):
        for s, sw in enumerate(_sub_widths(wv)):
            res[off:off + sw] = o[vg, :sw, s]
            off += sw
    return res


def _job_indices(fill_tokens_num, num_generation_jobs):
    fill = np.asarray(fill_tokens_num, dtype=np.int64)
    fill_last = np.cumsum(fill) - 1
    total_fill = int(fill.sum())
    gen = total_fill + np.arange(int(num_generation_jobs), dtype=np.int64)
    return np.concatenate([fill_last, gen])


def kernel(hidden_states, embd_weight, fill_tokens_num, num_generation_jobs):
    hs = np.asarray(hidden_states, dtype=np.float32)
    W = np.asarray(embd_weight, dtype=np.float32)
    V, Dd = W.shape

    idx = _job_indices(fill_tokens_num, num_generation_jobs)
    J = idx.size

    hs_sel = hs[idx]
    hst_host = np.ascontiguousarray(
        hs_sel.T.reshape(Dd // 256, 2, P, J).transpose(2, 0, 1, 3)
    ).astype(ml_dtypes.float8_e4m3)

    Wq = (W * W_SCALE).astype(ml_dtypes.float8_e4m3)
    WT_pad = np.zeros((Dd, V_PAD), dtype=ml_dtypes.float8_e4m3)
    WT_pad[:, :V] = Wq.T
    shards = [
        _pack_w(WT_pad[:, i * VS_EFF:(i + 1) * VS_EFF]) for i in range(N_CORES)
    ]

    nc = build_nc(J)
    kernel.last_nc = nc
    kernel.last_in_maps = [
        {"hst": hst_host, "wt": shards[i]} for i in range(N_CORES)
    ]
    res = run_bass_kernel_spmd(
        nc, kernel.last_in_maps, core_ids=list(range(N_CORES))
    )
    kernel.last_results = res

    logits = np.concatenate(
        [_decode_logits(res.results[i]["logits_t"], VGS, J)
         for i in range(N_CORES)],
        axis=0,
    ).T[:, :V]
    logits = np.where(np.isnan(logits), np.inf, logits)

    m = logits.max(axis=1, keepdims=True)
    rows, cols = np.nonzero(logits >= m - DELTA)
    exact = np.einsum(
        "ij,ij->i", hs_sel[rows].astype(np.float64), W[cols].astype(np.float64)
    )
    ids = np.zeros(J, dtype=np.int64)
    best = np.full(J, -np.inf)
    for r, c, s in zip(rows, cols, exact):
        if s > best[r]:
            best[r] = s
            ids[r] = c
    return ids.astype(np.int32)

